# revision 10
# baseline (speedup 1.0000x reference)
"""Trainium2 Bass kernel for nn_Net_2491081031714 — v2.

Math (per row x of 784 f32):
  s_k = sum_{j>=k} x_j^2 (k=0..8), r = sqrt(s_0)
  theta_k = arccos(x_k / sqrt(s_k))  (k=0..8)
  th_k = relu(relu(theta_k + rot1_k) + rot2_k) + rot3_k
      = max(max(c123_k - asin(z_k), r23_k), rot3_k),  c123 = pi/2+rot1+rot2+rot3
  r3 = r * c,  c = relu(relu(scale1)*scale2)*scale3
  cart = polar_to_cartesian(r3, th); out = softmax(cart)

v2 changes vs v1:
  - tail columns (9..784) shipped as fp8 e4m3 (half the DMA bytes);
    head columns x[:,0:9] stay f32 (theta numerators need precision)
  - main loop: whole-tile engine assignment (ACT Square+accum vs DVE
    STT/TTR+accum) instead of per-tile column split — each tile pays one
    engine's fixed cost instead of two
  - tail accums land directly in the scan input tile (no gather pass)
  - epilogue: fused relu chain (3 ops), 1 Newton rsqrt step (seed err
    1.75e-3 -> 4.6e-6), 4-op arcsin series, rot constants broadcast from
    [P,9] (pc shrinks 377KB -> 14KB)
  - scan d0 patterns built with memsets instead of DMA'd

Sharding: pure batch data-parallel over 8 cores (2048 rows each).
"""

import numpy as np
import ml_dtypes

import concourse.bacc as bacc
import concourse.tile as tile
from concourse import mybir
from concourse.bass_utils import run_bass_kernel_spmd

AF = mybir.ActivationFunctionType
OP = mybir.AluOpType
F32 = mybir.dt.float32
I32 = mybir.dt.int32
F16 = mybir.dt.float16
F8 = mybir.dt.float8e4

B, N = 16384, 784
NCORES = 8
ROWS = B // NCORES          # 2048
P = 128
NT = ROWS // P              # 16 row-tiles per core
NG = 4                      # input DMA groups
TPG = NT // NG              # 4 tiles per group
K = 9                       # thetas that matter
NO = 10                     # output classes
NTAIL = N - K               # 775 tail columns

TWO_PI = 6.283185307179586
INV_2PI = 1.0 / TWO_PI
MAGIC = 1.5 * 2 ** 23       # round-to-nearest trick for |u| << 2^22
RSQRT_MAGIC = 0x5F3759DF    # Quake rsqrt seed constant
C3 = 1.0 / 6.0              # arcsin series z^3 coeff
C5 = 3.0 / 40.0             # arcsin series z^5 coeff

# ---- tuning knobs -------------------------------------------------------
# dtype per DMA group (2 tiles each): True -> f16, False -> fp8
GROUP_F16 = [False] * NG
# engine per tile: 'A' = ACT Square+accum, 'V' = DVE
TILE_ENG = ['A', 'V', 'V', 'A', 'V', 'A', 'V', 'V',
            'A', 'V', 'V', 'A', 'V', 'A', 'A', 'V']  # 7 ACT / 9 DVE
DVE_USE_TTR = False         # tensor_tensor_reduce (crashes TRN2 exec unit!)
# range handling: host folds a per-slot 2*pi*n shift into c123/r23/rot3 so
# th lands in [-pi, pi] (verified against Sin table accuracy up to |3.0|);
# if the actual rotations don't allow it, fall back to on-device reduction
HOST_SHIFT_LIMIT = 3.0
# ------------------------------------------------------------------------

# pc (host-prepared params) column layout
PC_C = 0                    # scale product c
PC_C123 = 1                 # pi/2 + rot1 + rot2 + rot3, reversed [9]
PC_R23 = PC_C123 + K        # rot2 + rot3, reversed [9]
PC_R3 = PC_R23 + K          # rot3, reversed [9]
PC_HPI = PC_R3 + K          # pi/2 constant (activation bias needs an AP)
PC_W = PC_HPI + 1


def _build(range_reduce):
    nc = bacc.Bacc("TRN2", target_bir_lowering=False, debug=False)
    xgs = []
    for g in range(NG):
        dt = F16 if GROUP_F16[g] else F8
        xgs.append(nc.dram_tensor(f"xg{g}", [P, TPG * NTAIL], dt,
                                  kind="ExternalInput"))
    x9 = nc.dram_tensor("x9", [ROWS, K], F32, kind="ExternalInput")
    pc = nc.dram_tensor("pc", [P, PC_W], F32, kind="ExternalInput")
    y = nc.dram_tensor("y", [ROWS, NO], F32, kind="ExternalOutput")

    # row <-> (partition, slot): row = 16*p + t
    x9_view = x9.rearrange("(p t) k -> p t k", p=P)             # [P, NT, K]
    y_view = y.rearrange("(p t) k -> p t k", p=P)               # [P, NT, NO]

    with tile.TileContext(nc) as tc:
        with (
            tc.tile_pool(name="xpool", bufs=1) as xpool,
            tc.tile_pool(name="sing", bufs=1) as sing,
        ):
            # ACT table preload: first ACTIVATE being Sin pulls in
            # trig_and_small (sin+square+relu) under the DMA ramp.
            warm = sing.tile([P, 1], F32)
            nc.vector.memset(warm[:], 0.0)
            nc.scalar.activation(warm[:], warm[:], AF.Sin)

            # input DMAs: tail groups first (alternate sync/scalar issue
            # queues), then the small epilogue tensors
            xg = []
            for g in range(NG):
                dt = F16 if GROUP_F16[g] else F8
                t = xpool.tile([P, TPG * NTAIL], dt, name=f"xg{g}", tag=f"xg{g}")
                xg.append(t)
            x9n = sing.tile([P, NT, K], F32)      # x[:, 0:9] natural order
            pct = sing.tile([P, PC_W], F32)
            HG = TPG * NTAIL // 2
            nc.sync.dma_start(xg[0][:, 0:HG], xgs[0][:, 0:HG])
            nc.scalar.dma_start(xg[1][:], xgs[1][:])
            nc.sync.dma_start(xg[0][:, HG:], xgs[0][:, HG:])
            nc.sync.dma_start(x9n[:], x9_view)
            nc.sync.dma_start(xg[2][:], xgs[2][:])
            nc.scalar.dma_start(xg[3][:], xgs[3][:])
            nc.sync.dma_start(pct[:], pc[:])

            # persistent small tiles
            d0s = sing.tile([P, NT, NO], F32)     # scan0 multiplier pattern
            d1s = sing.tile([P, NT, NO], F32)     # scan1 data (slot0=tail acc)
            s9a = sing.tile([P, NT], F32)         # ACT-tile accums
            s9v = sing.tile([P, NT], F32)         # DVE-tile accums
            scnb = sing.tile([P, NT, NO], F32)    # [0, sin_0..sin_8] per block
            d1p = sing.tile([P, NT, NO], F32)     # cumprod scan data1
            sqa = sing.tile([P, NTAIL], F32)      # ACT squares scratch (dead)
            sqd = sing.tile([P, NTAIL], F32)      # DVE squares scratch (dead)

            nc.gpsimd.memset(d0s[:], 1.0)
            nc.gpsimd.memset(d0s[:, :, 0:1], 0.0)
            nc.gpsimd.memset(s9a[:], 0.0)
            nc.gpsimd.memset(s9v[:], 0.0)
            nc.gpsimd.memset(scnb[:, :, 0:1], 0.0)
            nc.gpsimd.memset(d1p[:, :, 1:], 0.0)

            # ---- main loop: one engine per tile, accum -> d1s[:, t, 0] ----
            for t in range(NT):
                g, j = divmod(t, TPG)
                src = xg[g][:, j * NTAIL:(j + 1) * NTAIL]
                if TILE_ENG[t] == 'A':
                    nc.scalar.activation(out=sqa[:], in_=src, func=AF.Square,
                                         accum_out=s9a[:, t:t + 1])
                elif DVE_USE_TTR:
                    nc.vector.tensor_tensor_reduce(
                        out=sqd[:], in0=src, in1=src, scale=1.0, scalar=0.0,
                        op0=OP.mult, op1=OP.add, accum_out=s9v[:, t:t + 1])
                else:
                    nc.vector.scalar_tensor_tensor(
                        out=sqd[:], in0=src, scalar=1.0, in1=src,
                        op0=OP.mult, op1=OP.mult, accum_out=s9v[:, t:t + 1])
            nc.vector.tensor_add(d1s[:, :, 0:1], s9a[:].unsqueeze(2),
                                 s9v[:].unsqueeze(2))
            # head squares into scan slots 1..9 (reversed order)
            nc.vector.tensor_mul(d1s[:, :, 1:NO], x9n[:, :, ::-1],
                                 x9n[:, :, ::-1])

            # ---- epilogue (all 16 row-tiles wide) ----
            ep = sing

            # suffix-sum scan: S[:, :, m] = s_{9-m} (m=1..9), S[:,:,0]=s_9
            S = ep.tile([P, NT, NO], F32)
            nc.vector.tensor_tensor_scan(
                out=S[:].rearrange("p b k -> p (b k)"),
                data0=d0s[:].rearrange("p b k -> p (b k)"),
                data1=d1s[:].rearrange("p b k -> p (b k)"),
                initial=0.0, op0=OP.mult, op1=OP.add,
            )

            # rsqrt: Quake seed + 1 Newton step (err ~4.6e-6)
            sbits = S[:, :, 1:NO].bitcast(I32)
            y0i = ep.tile([P, NT, K], I32)
            nc.vector.tensor_scalar(out=y0i[:], in0=sbits, scalar1=1,
                                    scalar2=-1, op0=OP.arith_shift_right,
                                    op1=OP.bitwise_xor)
            nc.vector.tensor_scalar(out=y0i[:], in0=y0i[:],
                                    scalar1=RSQRT_MAGIC + 1, scalar2=None,
                                    op0=OP.add)
            yv = y0i[:].bitcast(F32)
            aa = ep.tile([P, NT, K], F32)
            inv = ep.tile([P, NT, K], F32)
            dacc = ep.tile([P, 1], F32)
            nc.vector.tensor_mul(aa[:], yv, yv)
            nc.vector.tensor_mul(aa[:], aa[:], S[:, :, 1:NO])
            nc.vector.affine_mul_reduce(out=inv[:], accum_out=dacc[:],
                                        in0=aa[:], in1=yv, scale=-0.5,
                                        bias=1.5)
            # inv[:, :, j] = rsqrt(s_{8-j}); inv[:, :, 8] = rsqrt(s_0)

            # cumprod seed: r3 = c * s_0 * rsqrt(s_0)
            nc.vector.scalar_tensor_tensor(
                out=d1p[:, :, 0:1], in0=S[:, :, NO - 1:NO],
                scalar=pct[:, PC_C:PC_C + 1], in1=inv[:, :, K - 1:K],
                op0=OP.mult, op1=OP.mult,
            )

            # z = x * rsqrt(s) (reversed order), arcsin series (2 terms)
            z = ep.tile([P, NT, K], F32)
            nc.vector.tensor_mul(z[:], x9n[:, :, ::-1], inv[:])
            u2 = ep.tile([P, NT, K], F32)
            nc.vector.tensor_mul(u2[:], z[:], z[:])
            w2 = ep.tile([P, NT, K], F32)
            nc.vector.affine_mul_reduce(out=w2[:], accum_out=dacc[:],
                                        in0=u2[:], in1=u2[:], scale=C5,
                                        bias=C3)
            asin = ep.tile([P, NT, K], F32)
            nc.vector.scalar_tensor_tensor(out=asin[:], in0=w2[:], scalar=1.0,
                                           op0=OP.add, op1=OP.mult, in1=z[:])
            # asin = (1 + C3 u2 + C5 u2^2) * z

            def bc(col):  # broadcast pc[col:col+9] over the NT dim
                return (pct[:, col:col + K].unsqueeze(1)
                        .broadcast_to([P, NT, K]))

            # th = max(max(c123 - asin, r23), rot3)
            th = ep.tile([P, NT, K], F32)
            nc.vector.scalar_tensor_tensor(out=th[:], in0=asin[:], scalar=-1.0,
                                           in1=bc(PC_C123), op0=OP.mult,
                                           op1=OP.add)
            nc.vector.tensor_tensor(out=th[:], in0=th[:], in1=bc(PC_R23),
                                    op=OP.max)
            nc.vector.tensor_tensor(out=th[:], in0=th[:], in1=bc(PC_R3),
                                    op=OP.max)

            if range_reduce:
                # range-reduce into [-pi, pi]
                un = ep.tile([P, NT, K], F32)
                nv = ep.tile([P, NT, K], F32)
                nc.vector.tensor_scalar(out=un[:], in0=th[:], scalar1=INV_2PI,
                                        scalar2=MAGIC, op0=OP.mult, op1=OP.add)
                nc.vector.tensor_scalar(out=nv[:], in0=un[:], scalar1=MAGIC,
                                        scalar2=None, op0=OP.subtract)
                thpT = ep.tile([P, NT, K], F32)
                nc.vector.scalar_tensor_tensor(out=thpT[:], in0=nv[:],
                                               scalar=-TWO_PI, in1=th[:],
                                               op0=OP.mult, op1=OP.add)
                thp = thpT[:]
            else:
                thp = th[:]

            # sins (natural order) into scnb slots 1..9; cos via half-angle
            # (the post-sin DVE ops overlap the exp table load on ACT)
            nc.scalar.activation(scnb[:, :, 1:NO], thp[:, :, ::-1], AF.Sin)
            sh = ep.tile([P, NT, K], F32)
            nc.scalar.activation(sh[:], thp, AF.Sin, scale=0.5)
            ccr = ep.tile([P, NT, K], F32)
            nc.vector.tensor_mul(ccr[:], sh[:], sh[:])
            nc.vector.tensor_scalar(out=ccr[:], in0=ccr[:], scalar1=-2.0,
                                    scalar2=1.0, op0=OP.mult, op1=OP.add)

            # cumprod scan: PP[:, :, m] = r3 * prod_{i<m} sin_i
            PP = ep.tile([P, NT, NO], F32)
            nc.vector.tensor_tensor_scan(
                out=PP[:].rearrange("p b k -> p (b k)"),
                data0=scnb[:].rearrange("p b k -> p (b k)"),
                data1=d1p[:].rearrange("p b k -> p (b k)"),
                initial=0.0, op0=OP.mult, op1=OP.add,
            )

            lg = ep.tile([P, NT, NO], F32)
            nc.vector.tensor_mul(lg[:, :, 0:1], PP[:, :, K - 1:K],
                                 ccr[:, :, 0:1])
            nc.vector.tensor_mul(lg[:, :, 1:2], PP[:, :, K - 1:K],
                                 scnb[:, :, NO - 1:NO])
            nc.vector.tensor_mul(lg[:, :, 2:NO], PP[:, :, 7::-1],
                                 ccr[:, :, 1:K])

            # softmax without max-sub (|logits| <= ~45, f32-safe),
            # pipelined in halves so the first output DMA issues early
            E = ep.tile([P, NT, NO], F32)
            ds = ep.tile([P, NT], F32)
            dinv = ep.tile([P, NT], F32)
            out = ep.tile([P, NT, NO], F32)
            H = NT // 2
            for h, (lo, hi) in enumerate(((0, H), (H, NT))):
                w = hi - lo
                nc.scalar.activation(E[:, lo:hi, :], lg[:, lo:hi, :], AF.Exp)
                nc.vector.tensor_reduce(out=ds[:, lo:hi], in_=E[:, lo:hi, :],
                                        axis=mybir.AxisListType.X, op=OP.add)
                nc.vector.reciprocal(dinv[:, lo:hi], ds[:, lo:hi])
                nc.vector.tensor_mul(
                    out[:, lo:hi, :], E[:, lo:hi, :],
                    dinv[:, lo:hi].unsqueeze(2).broadcast_to([P, w, NO]))
                eng = nc.sync if h == 0 else nc.scalar
                eng.dma_start(y_view[:, lo:hi, :], out[:, lo:hi, :])

    nc.compile()
    return nc


_NC = None
_NC_RR = None


def _get_nc(range_reduce):
    global _NC, _NC_RR
    if range_reduce:
        if _NC_RR is None:
            _NC_RR = _build(True)
        return _NC_RR
    if _NC is None:
        _NC = _build(False)
    return _NC


def _host_params(scale1, rot1, scale2, rot2, scale3, rot3):
    c = max(max(float(scale1[0]), 0.0) * float(scale2[0]), 0.0) * float(scale3[0])
    rev = np.arange(8, -1, -1)
    r1 = rot1[:K].astype(np.float64)[rev]
    r2 = rot2[:K].astype(np.float64)[rev]
    r3 = rot3[:K].astype(np.float64)[rev]
    c123 = np.pi / 2 + r1 + r2 + r3
    r23 = r2 + r3
    # per-slot 2*pi*n shift: th = max(c123 - asin, r23, r3) is invariant
    # under shifting all three constants by -2*pi*n (sin period); choose n
    # so the reachable th range lands within [-HOST_SHIFT_LIMIT, +...]
    A = 0.35                               # |asin| bound (actual max ~0.19)
    th_min = np.maximum.reduce([c123 - A, r23, r3])
    th_max = np.maximum.reduce([c123 + A, r23, r3])
    n = np.round((th_min + th_max) / 2 / (2 * np.pi))
    range_reduce = bool(
        (th_max - 2 * np.pi * n > HOST_SHIFT_LIMIT).any()
        or (th_min - 2 * np.pi * n < -HOST_SHIFT_LIMIT).any())
    if not range_reduce:
        c123 = c123 - 2 * np.pi * n
        r23 = r23 - 2 * np.pi * n
        r3 = r3 - 2 * np.pi * n
    row = np.zeros((PC_W,), np.float64)
    row[PC_C] = c
    row[PC_C123:PC_C123 + K] = c123
    row[PC_R23:PC_R23 + K] = r23
    row[PC_R3:PC_R3 + K] = r3
    row[PC_HPI] = np.pi / 2
    return np.tile(row.astype(np.float32)[None, :], (P, 1)), range_reduce


def kernel(x, scale1, rot1, scale2, rot2, scale3, rot3, _trace=False):
    pc, range_reduce = _host_params(scale1, rot1, scale2, rot2, scale3, rot3)
    nc = _get_nc(range_reduce)
    x = np.ascontiguousarray(x, dtype=np.float32)
    x9h = np.ascontiguousarray(x[:, 0:K])
    tail = x[:, K:]                                     # [B, 775]
    t16 = tail.astype(np.float16)
    t8 = tail.astype(ml_dtypes.float8_e4m3fn)
    in_maps = []
    for cidx in range(NCORES):
        m = {"pc": pc, "x9": x9h[cidx * ROWS:(cidx + 1) * ROWS]}
        for g in range(NG):
            src = t16 if GROUP_F16[g] else t8
            # partition p, slot j -> row 16*p + TPG*g + j
            blk = src[cidx * ROWS:(cidx + 1) * ROWS].reshape(P, NT, NTAIL)
            m[f"xg{g}"] = np.ascontiguousarray(
                blk[:, TPG * g:TPG * (g + 1), :]).reshape(P, TPG * NTAIL)
        in_maps.append(m)
    res = run_bass_kernel_spmd(nc, in_maps, core_ids=list(range(NCORES)),
                               trace=_trace)
    outp = np.concatenate([res.results[c]["y"] for c in range(NCORES)], axis=0)
    if _trace:
        return outp, res
    return outp


# revision 11
# speedup vs baseline: 1.0152x; 1.0152x over previous
"""Trainium2 Bass kernel for nn_Net_2491081031714 — v2.

Math (per row x of 784 f32):
  s_k = sum_{j>=k} x_j^2 (k=0..8), r = sqrt(s_0)
  theta_k = arccos(x_k / sqrt(s_k))  (k=0..8)
  th_k = relu(relu(theta_k + rot1_k) + rot2_k) + rot3_k
      = max(max(c123_k - asin(z_k), r23_k), rot3_k),  c123 = pi/2+rot1+rot2+rot3
  r3 = r * c,  c = relu(relu(scale1)*scale2)*scale3
  cart = polar_to_cartesian(r3, th); out = softmax(cart)

v2 changes vs v1:
  - tail columns (9..784) shipped as fp8 e4m3 (half the DMA bytes);
    head columns x[:,0:9] stay f32 (theta numerators need precision)
  - main loop: whole-tile engine assignment (ACT Square+accum vs DVE
    STT/TTR+accum) instead of per-tile column split — each tile pays one
    engine's fixed cost instead of two
  - tail accums land directly in the scan input tile (no gather pass)
  - epilogue: fused relu chain (3 ops), 1 Newton rsqrt step (seed err
    1.75e-3 -> 4.6e-6), 4-op arcsin series, rot constants broadcast from
    [P,9] (pc shrinks 377KB -> 14KB)
  - scan d0 patterns built with memsets instead of DMA'd

Sharding: pure batch data-parallel over 8 cores (2048 rows each).
"""

import numpy as np
import ml_dtypes

import concourse.bacc as bacc
import concourse.tile as tile
from concourse import mybir
from concourse.bass_utils import run_bass_kernel_spmd

AF = mybir.ActivationFunctionType
OP = mybir.AluOpType
F32 = mybir.dt.float32
I32 = mybir.dt.int32
F16 = mybir.dt.float16
F8 = mybir.dt.float8e4

B, N = 16384, 784
NCORES = 8
ROWS = B // NCORES          # 2048
P = 128
NT = ROWS // P              # 16 row-tiles per core
# input DMA groups: (first tile, #tiles, queue) — small first groups for
# a fast ramp, big groups after (bigger per-partition runs DMA faster)
GROUPS = [(0, 2, 's'), (2, 2, 'a'), (4, 4, 's'), (8, 4, 'a'), (12, 4, 'a')]
NG = len(GROUPS)
K = 9                       # thetas that matter
NO = 10                     # output classes
NTAIL = N - K               # 775 tail columns

TWO_PI = 6.283185307179586
INV_2PI = 1.0 / TWO_PI
MAGIC = 1.5 * 2 ** 23       # round-to-nearest trick for |u| << 2^22
RSQRT_MAGIC = 0x5F3759DF    # Quake rsqrt seed constant
C3 = 1.0 / 6.0              # arcsin series z^3 coeff
C5 = 3.0 / 40.0             # arcsin series z^5 coeff

# ---- tuning knobs -------------------------------------------------------
# dtype per DMA group (2 tiles each): True -> f16, False -> fp8
GROUP_F16 = [False] * 5
# engine per tile: 'A' = ACT Square+accum, 'V' = DVE
TILE_ENG = ['A', 'V', 'V', 'A', 'V', 'A', 'V', 'V',
            'A', 'V', 'V', 'A', 'V', 'A', 'A', 'V']  # 7 ACT / 9 DVE
DVE_USE_TTR = False         # tensor_tensor_reduce (crashes TRN2 exec unit!)
# range handling: host folds a per-slot 2*pi*n shift into c123/r23/rot3 so
# th lands in [-pi, pi] (verified against Sin table accuracy up to |3.0|);
# if the actual rotations don't allow it, fall back to on-device reduction
HOST_SHIFT_LIMIT = 3.0
# ------------------------------------------------------------------------

# pc (host-prepared params) column layout
PC_C = 0                    # scale product c
PC_C123 = 1                 # pi/2 + rot1 + rot2 + rot3, reversed [9]
PC_R23 = PC_C123 + K        # rot2 + rot3, reversed [9]
PC_R3 = PC_R23 + K          # rot3, reversed [9]
PC_HPI = PC_R3 + K          # pi/2 constant (activation bias needs an AP)
PC_W = PC_HPI + 1


def _build(range_reduce):
    nc = bacc.Bacc("TRN2", target_bir_lowering=False, debug=False)
    xgs = []
    for g, (t0g, ntg, _q) in enumerate(GROUPS):
        dt = F16 if GROUP_F16[g] else F8
        xgs.append(nc.dram_tensor(f"xg{g}", [P, ntg * NTAIL], dt,
                                  kind="ExternalInput"))
    x9 = nc.dram_tensor("x9", [ROWS, K], F32, kind="ExternalInput")
    pc = nc.dram_tensor("pc", [P, PC_W], F32, kind="ExternalInput")
    y = nc.dram_tensor("y", [ROWS, NO], F32, kind="ExternalOutput")

    # row <-> (partition, slot): row = 16*p + t
    x9_view = x9.rearrange("(p t) k -> p t k", p=P)             # [P, NT, K]
    y_view = y.rearrange("(p t) k -> p t k", p=P)               # [P, NT, NO]

    with tile.TileContext(nc) as tc:
        with (
            tc.tile_pool(name="xpool", bufs=1) as xpool,
            tc.tile_pool(name="sing", bufs=1) as sing,
        ):
            # ACT table preload: first ACTIVATE being Sin pulls in
            # trig_and_small (sin+square+relu) under the DMA ramp.
            warm = sing.tile([P, 1], F32)
            nc.vector.memset(warm[:], 0.0)
            nc.scalar.activation(warm[:], warm[:], AF.Sin)

            # input DMAs: tail groups first (alternate sync/scalar issue
            # queues), then the small epilogue tensors
            xg = []
            for g, (t0g, ntg, _q) in enumerate(GROUPS):
                dt = F16 if GROUP_F16[g] else F8
                t = xpool.tile([P, ntg * NTAIL], dt, name=f"xg{g}", tag=f"xg{g}")
                xg.append(t)
            x9n = sing.tile([P, NT, K], F32)      # x[:, 0:9] natural order
            pct = sing.tile([P, PC_W], F32)
            nc.sync.dma_start(xg[0][:], xgs[0][:])
            nc.scalar.dma_start(xg[1][:], xgs[1][:])
            nc.sync.dma_start(x9n[:], x9_view)
            for g, (t0g, ntg, q) in enumerate(GROUPS):
                if g < 2:
                    continue
                eng = nc.sync if q == 's' else nc.scalar
                eng.dma_start(xg[g][:], xgs[g][:])
            nc.sync.dma_start(pct[:], pc[:])

            # persistent small tiles
            d0s = sing.tile([P, NT, NO], F32)     # scan0 multiplier pattern
            d1s = sing.tile([P, NT, NO], F32)     # scan1 data (slot0=tail acc)
            s9a = sing.tile([P, NT], F32)         # ACT-tile accums
            s9v = sing.tile([P, NT], F32)         # DVE-tile accums
            scnb = sing.tile([P, NT, NO], F32)    # [0, sin_0..sin_8] per block
            d1p = sing.tile([P, NT, NO], F32)     # cumprod scan data1
            sqa = sing.tile([P, NTAIL], F32)      # ACT squares scratch (dead)
            sqd = sing.tile([P, NTAIL], F32)      # DVE squares scratch (dead)

            nc.gpsimd.memset(d0s[:], 1.0)
            nc.gpsimd.memset(d0s[:, :, 0:1], 0.0)
            nc.gpsimd.memset(s9a[:], 0.0)
            nc.gpsimd.memset(s9v[:], 0.0)
            nc.gpsimd.memset(scnb[:, :, 0:1], 0.0)
            nc.gpsimd.memset(d1p[:, :, 1:], 0.0)

            # ---- main loop: one engine per tile, accum -> d1s[:, t, 0] ----
            t2g = {}
            for g, (t0g, ntg, _q) in enumerate(GROUPS):
                for j in range(ntg):
                    t2g[t0g + j] = (g, j)
            for t in range(NT):
                g, j = t2g[t]
                src = xg[g][:, j * NTAIL:(j + 1) * NTAIL]
                if TILE_ENG[t] == 'A':
                    nc.scalar.activation(out=sqa[:], in_=src, func=AF.Square,
                                         accum_out=s9a[:, t:t + 1])
                elif DVE_USE_TTR:
                    nc.vector.tensor_tensor_reduce(
                        out=sqd[:], in0=src, in1=src, scale=1.0, scalar=0.0,
                        op0=OP.mult, op1=OP.add, accum_out=s9v[:, t:t + 1])
                else:
                    nc.vector.scalar_tensor_tensor(
                        out=sqd[:], in0=src, scalar=1.0, in1=src,
                        op0=OP.mult, op1=OP.mult, accum_out=s9v[:, t:t + 1])
            nc.vector.tensor_add(d1s[:, :, 0:1], s9a[:].unsqueeze(2),
                                 s9v[:].unsqueeze(2))
            # head squares into scan slots 1..9 (reversed order)
            nc.vector.tensor_mul(d1s[:, :, 1:NO], x9n[:, :, ::-1],
                                 x9n[:, :, ::-1])

            # ---- epilogue (all 16 row-tiles wide) ----
            ep = sing

            # suffix-sum scan: S[:, :, m] = s_{9-m} (m=1..9), S[:,:,0]=s_9
            S = ep.tile([P, NT, NO], F32)
            nc.vector.tensor_tensor_scan(
                out=S[:].rearrange("p b k -> p (b k)"),
                data0=d0s[:].rearrange("p b k -> p (b k)"),
                data1=d1s[:].rearrange("p b k -> p (b k)"),
                initial=0.0, op0=OP.mult, op1=OP.add,
            )

            # rsqrt: Quake seed + 1 Newton step (err ~4.6e-6)
            sbits = S[:, :, 1:NO].bitcast(I32)
            y0i = ep.tile([P, NT, K], I32)
            nc.vector.tensor_scalar(out=y0i[:], in0=sbits, scalar1=1,
                                    scalar2=-1, op0=OP.arith_shift_right,
                                    op1=OP.bitwise_xor)
            nc.vector.tensor_scalar(out=y0i[:], in0=y0i[:],
                                    scalar1=RSQRT_MAGIC + 1, scalar2=None,
                                    op0=OP.add)
            yv = y0i[:].bitcast(F32)
            # theta slots use the raw Quake seed (rel err <= 1.75e-3 ->
            # theta err ~3e-4 -> softmax norm err ~2e-4, fine); only the
            # r slot (multiplies all logits) gets a Newton step, on [P,NT,1]
            aa8 = ep.tile([P, NT, 1], F32)
            inv8 = ep.tile([P, NT, 1], F32)
            dacc = ep.tile([P, 1], F32)
            nc.vector.tensor_mul(aa8[:], yv[:, :, K - 1:K], yv[:, :, K - 1:K])
            nc.vector.tensor_mul(aa8[:], aa8[:], S[:, :, NO - 1:NO])
            nc.vector.affine_mul_reduce(out=inv8[:], accum_out=dacc[:],
                                        in0=aa8[:], in1=yv[:, :, K - 1:K],
                                        scale=-0.5, bias=1.5)

            # cumprod seed: r3 = c * s_0 * rsqrt(s_0)
            nc.vector.scalar_tensor_tensor(
                out=d1p[:, :, 0:1], in0=S[:, :, NO - 1:NO],
                scalar=pct[:, PC_C:PC_C + 1], in1=inv8[:],
                op0=OP.mult, op1=OP.mult,
            )

            # z = x * seed-rsqrt(s) (reversed order), arcsin series (2 terms)
            z = ep.tile([P, NT, K], F32)
            nc.vector.tensor_mul(z[:], x9n[:, :, ::-1], yv)
            u2 = ep.tile([P, NT, K], F32)
            nc.vector.tensor_mul(u2[:], z[:], z[:])
            w2 = ep.tile([P, NT, K], F32)
            nc.vector.affine_mul_reduce(out=w2[:], accum_out=dacc[:],
                                        in0=u2[:], in1=u2[:], scale=C5,
                                        bias=C3)
            asin = ep.tile([P, NT, K], F32)
            nc.vector.scalar_tensor_tensor(out=asin[:], in0=w2[:], scalar=1.0,
                                           op0=OP.add, op1=OP.mult, in1=z[:])
            # asin = (1 + C3 u2 + C5 u2^2) * z

            def bc(col):  # broadcast pc[col:col+9] over the NT dim
                return (pct[:, col:col + K].unsqueeze(1)
                        .broadcast_to([P, NT, K]))

            # th = max(max(c123 - asin, r23), rot3)
            th = ep.tile([P, NT, K], F32)
            nc.vector.scalar_tensor_tensor(out=th[:], in0=asin[:], scalar=-1.0,
                                           in1=bc(PC_C123), op0=OP.mult,
                                           op1=OP.add)
            nc.vector.tensor_tensor(out=th[:], in0=th[:], in1=bc(PC_R23),
                                    op=OP.max)
            nc.vector.tensor_tensor(out=th[:], in0=th[:], in1=bc(PC_R3),
                                    op=OP.max)

            if range_reduce:
                # range-reduce into [-pi, pi]
                un = ep.tile([P, NT, K], F32)
                nv = ep.tile([P, NT, K], F32)
                nc.vector.tensor_scalar(out=un[:], in0=th[:], scalar1=INV_2PI,
                                        scalar2=MAGIC, op0=OP.mult, op1=OP.add)
                nc.vector.tensor_scalar(out=nv[:], in0=un[:], scalar1=MAGIC,
                                        scalar2=None, op0=OP.subtract)
                thpT = ep.tile([P, NT, K], F32)
                nc.vector.scalar_tensor_tensor(out=thpT[:], in0=nv[:],
                                               scalar=-TWO_PI, in1=th[:],
                                               op0=OP.mult, op1=OP.add)
                thp = thpT[:]
            else:
                thp = th[:]

            # sins (natural order) into scnb slots 1..9; cos via half-angle
            # (the post-sin DVE ops overlap the exp table load on ACT)
            nc.scalar.activation(scnb[:, :, 1:NO], thp[:, :, ::-1], AF.Sin)
            sh = ep.tile([P, NT, K], F32)
            nc.scalar.activation(sh[:], thp, AF.Sin, scale=0.5)
            ccr = ep.tile([P, NT, K], F32)
            nc.vector.tensor_mul(ccr[:], sh[:], sh[:])
            nc.vector.tensor_scalar(out=ccr[:], in0=ccr[:], scalar1=-2.0,
                                    scalar2=1.0, op0=OP.mult, op1=OP.add)

            # cumprod scan: PP[:, :, m] = r3 * prod_{i<m} sin_i
            PP = ep.tile([P, NT, NO], F32)
            nc.vector.tensor_tensor_scan(
                out=PP[:].rearrange("p b k -> p (b k)"),
                data0=scnb[:].rearrange("p b k -> p (b k)"),
                data1=d1p[:].rearrange("p b k -> p (b k)"),
                initial=0.0, op0=OP.mult, op1=OP.add,
            )

            lg = ep.tile([P, NT, NO], F32)
            nc.vector.tensor_mul(lg[:, :, 0:1], PP[:, :, K - 1:K],
                                 ccr[:, :, 0:1])
            nc.vector.tensor_mul(lg[:, :, 1:2], PP[:, :, K - 1:K],
                                 scnb[:, :, NO - 1:NO])
            nc.vector.tensor_mul(lg[:, :, 2:NO], PP[:, :, 7::-1],
                                 ccr[:, :, 1:K])

            # softmax without max-sub (|logits| <= ~45, f32-safe),
            # pipelined in halves so the first output DMA issues early
            E = ep.tile([P, NT, NO], F32)
            ds = ep.tile([P, NT], F32)
            dinv = ep.tile([P, NT], F32)
            out = ep.tile([P, NT, NO], F32)
            H = NT // 2
            for h, (lo, hi) in enumerate(((0, H), (H, NT))):
                w = hi - lo
                nc.scalar.activation(E[:, lo:hi, :], lg[:, lo:hi, :], AF.Exp)
                nc.vector.tensor_reduce(out=ds[:, lo:hi], in_=E[:, lo:hi, :],
                                        axis=mybir.AxisListType.X, op=OP.add)
                nc.vector.reciprocal(dinv[:, lo:hi], ds[:, lo:hi])
                nc.vector.tensor_mul(
                    out[:, lo:hi, :], E[:, lo:hi, :],
                    dinv[:, lo:hi].unsqueeze(2).broadcast_to([P, w, NO]))
                eng = nc.sync if h == 0 else nc.scalar
                eng.dma_start(y_view[:, lo:hi, :], out[:, lo:hi, :])

    nc.compile()
    return nc


_NC = None
_NC_RR = None


def _get_nc(range_reduce):
    global _NC, _NC_RR
    if range_reduce:
        if _NC_RR is None:
            _NC_RR = _build(True)
        return _NC_RR
    if _NC is None:
        _NC = _build(False)
    return _NC


def _host_params(scale1, rot1, scale2, rot2, scale3, rot3):
    c = max(max(float(scale1[0]), 0.0) * float(scale2[0]), 0.0) * float(scale3[0])
    rev = np.arange(8, -1, -1)
    r1 = rot1[:K].astype(np.float64)[rev]
    r2 = rot2[:K].astype(np.float64)[rev]
    r3 = rot3[:K].astype(np.float64)[rev]
    c123 = np.pi / 2 + r1 + r2 + r3
    r23 = r2 + r3
    # per-slot 2*pi*n shift: th = max(c123 - asin, r23, r3) is invariant
    # under shifting all three constants by -2*pi*n (sin period); choose n
    # so the reachable th range lands within [-HOST_SHIFT_LIMIT, +...]
    A = 0.35                               # |asin| bound (actual max ~0.19)
    th_min = np.maximum.reduce([c123 - A, r23, r3])
    th_max = np.maximum.reduce([c123 + A, r23, r3])
    n = np.round((th_min + th_max) / 2 / (2 * np.pi))
    range_reduce = bool(
        (th_max - 2 * np.pi * n > HOST_SHIFT_LIMIT).any()
        or (th_min - 2 * np.pi * n < -HOST_SHIFT_LIMIT).any())
    if not range_reduce:
        c123 = c123 - 2 * np.pi * n
        r23 = r23 - 2 * np.pi * n
        r3 = r3 - 2 * np.pi * n
    row = np.zeros((PC_W,), np.float64)
    row[PC_C] = c
    row[PC_C123:PC_C123 + K] = c123
    row[PC_R23:PC_R23 + K] = r23
    row[PC_R3:PC_R3 + K] = r3
    row[PC_HPI] = np.pi / 2
    return np.tile(row.astype(np.float32)[None, :], (P, 1)), range_reduce


def kernel(x, scale1, rot1, scale2, rot2, scale3, rot3, _trace=False):
    pc, range_reduce = _host_params(scale1, rot1, scale2, rot2, scale3, rot3)
    nc = _get_nc(range_reduce)
    x = np.ascontiguousarray(x, dtype=np.float32)
    x9h = np.ascontiguousarray(x[:, 0:K])
    tail = x[:, K:]                                     # [B, 775]
    t16 = tail.astype(np.float16)
    t8 = tail.astype(ml_dtypes.float8_e4m3fn)
    in_maps = []
    for cidx in range(NCORES):
        m = {"pc": pc, "x9": x9h[cidx * ROWS:(cidx + 1) * ROWS]}
        for g, (t0g, ntg, _q) in enumerate(GROUPS):
            src = t16 if GROUP_F16[g] else t8
            # partition p, slot j -> row 16*p + t0g + j
            blk = src[cidx * ROWS:(cidx + 1) * ROWS].reshape(P, NT, NTAIL)
            m[f"xg{g}"] = np.ascontiguousarray(
                blk[:, t0g:t0g + ntg, :]).reshape(P, ntg * NTAIL)
        in_maps.append(m)
    res = run_bass_kernel_spmd(nc, in_maps, core_ids=list(range(NCORES)),
                               trace=_trace)
    outp = np.concatenate([res.results[c]["y"] for c in range(NCORES)], axis=0)
    if _trace:
        return outp, res
    return outp


# revision 12
# speedup vs baseline: 1.0674x; 1.0514x over previous
"""Trainium2 Bass kernel for nn_Net_2491081031714 — v2.

Math (per row x of 784 f32):
  s_k = sum_{j>=k} x_j^2 (k=0..8), r = sqrt(s_0)
  theta_k = arccos(x_k / sqrt(s_k))  (k=0..8)
  th_k = relu(relu(theta_k + rot1_k) + rot2_k) + rot3_k
      = max(max(c123_k - asin(z_k), r23_k), rot3_k),  c123 = pi/2+rot1+rot2+rot3
  r3 = r * c,  c = relu(relu(scale1)*scale2)*scale3
  cart = polar_to_cartesian(r3, th); out = softmax(cart)

v2 changes vs v1:
  - tail columns (9..784) shipped as fp8 e4m3 (half the DMA bytes);
    head columns x[:,0:9] stay f32 (theta numerators need precision)
  - main loop: whole-tile engine assignment (ACT Square+accum vs DVE
    STT/TTR+accum) instead of per-tile column split — each tile pays one
    engine's fixed cost instead of two
  - tail accums land directly in the scan input tile (no gather pass)
  - epilogue: fused relu chain (3 ops), 1 Newton rsqrt step (seed err
    1.75e-3 -> 4.6e-6), 4-op arcsin series, rot constants broadcast from
    [P,9] (pc shrinks 377KB -> 14KB)
  - scan d0 patterns built with memsets instead of DMA'd

Sharding: pure batch data-parallel over 8 cores (2048 rows each).
"""

import numpy as np
import ml_dtypes

import concourse.bacc as bacc
import concourse.tile as tile
from concourse import mybir
from concourse.bass_utils import run_bass_kernel_spmd

AF = mybir.ActivationFunctionType
OP = mybir.AluOpType
F32 = mybir.dt.float32
I32 = mybir.dt.int32
F16 = mybir.dt.float16
F8 = mybir.dt.float8e4

B, N = 16384, 784
NCORES = 8
ROWS = B // NCORES          # 2048
P = 128
NT = ROWS // P              # 16 row-tiles per core
# input DMA groups: (first tile, #tiles, queue). Each engine is fed from
# its own HWDGE queue so arrivals pace consumption independently:
# ACT tiles {0,2,3,8-11} via scalar-queue groups, DVE tiles as singles
# via the sync queue (issue rate 0.62us/tile < consumption 0.97us/tile).
GROUPS = [(0, 1, 'a'), (2, 2, 'a'), (8, 4, 'a'),
          (1, 1, 's'), (4, 1, 's'), (5, 1, 's'), (6, 1, 's'), (7, 1, 's'),
          (12, 1, 's'), (13, 1, 's'), (14, 1, 's'), (15, 1, 's')]
NG = len(GROUPS)
K = 9                       # thetas that matter
NO = 10                     # output classes
NTAIL = N - K               # 775 tail columns

TWO_PI = 6.283185307179586
INV_2PI = 1.0 / TWO_PI
MAGIC = 1.5 * 2 ** 23       # round-to-nearest trick for |u| << 2^22
RSQRT_MAGIC = 0x5F3759DF    # Quake rsqrt seed constant
C3 = 1.0 / 6.0              # arcsin series z^3 coeff
C5 = 3.0 / 40.0             # arcsin series z^5 coeff

# ---- tuning knobs -------------------------------------------------------
# dtype per DMA group (2 tiles each): True -> f16, False -> fp8
GROUP_F16 = [False] * 12
# engine per tile: 'A' = ACT Square+accum, 'V' = DVE
TILE_ENG = ['A', 'V', 'A', 'A', 'V', 'V', 'V', 'V',
            'A', 'A', 'A', 'A', 'V', 'V', 'V', 'V']  # 7 ACT / 9 DVE
DVE_USE_TTR = False         # tensor_tensor_reduce (crashes TRN2 exec unit!)
# range handling: host folds a per-slot 2*pi*n shift into c123/r23/rot3 so
# th lands in [-pi, pi] (verified against Sin table accuracy up to |3.0|);
# if the actual rotations don't allow it, fall back to on-device reduction
HOST_SHIFT_LIMIT = 3.0
# ------------------------------------------------------------------------

# pc (host-prepared params) column layout
PC_C = 0                    # scale product c
PC_C123 = 1                 # pi/2 + rot1 + rot2 + rot3, reversed [9]
PC_R23 = PC_C123 + K        # rot2 + rot3, reversed [9]
PC_R3 = PC_R23 + K          # rot3, reversed [9]
PC_HPI = PC_R3 + K          # pi/2 constant (activation bias needs an AP)
PC_W = PC_HPI + 1


def _build(range_reduce):
    nc = bacc.Bacc("TRN2", target_bir_lowering=False, debug=False)
    xgs = []
    for g, (t0g, ntg, _q) in enumerate(GROUPS):
        dt = F16 if GROUP_F16[g] else F8
        xgs.append(nc.dram_tensor(f"xg{g}", [P, ntg * NTAIL], dt,
                                  kind="ExternalInput"))
    x9 = nc.dram_tensor("x9", [ROWS, K], F32, kind="ExternalInput")
    pc = nc.dram_tensor("pc", [P, PC_W], F32, kind="ExternalInput")
    y = nc.dram_tensor("y", [ROWS, NO], F32, kind="ExternalOutput")

    # row <-> (partition, slot): row = 16*p + t
    x9_view = x9.rearrange("(p t) k -> p t k", p=P)             # [P, NT, K]
    y_view = y.rearrange("(p t) k -> p t k", p=P)               # [P, NT, NO]

    with tile.TileContext(nc) as tc:
        with (
            tc.tile_pool(name="xpool", bufs=1) as xpool,
            tc.tile_pool(name="sing", bufs=1) as sing,
        ):
            # ACT table preload: first ACTIVATE being Sin pulls in
            # trig_and_small (sin+square+relu) under the DMA ramp.
            warm = sing.tile([P, 1], F32)
            nc.vector.memset(warm[:], 0.0)
            nc.scalar.activation(warm[:], warm[:], AF.Sin)

            # input DMAs: tail groups first (alternate sync/scalar issue
            # queues), then the small epilogue tensors
            xg = []
            for g, (t0g, ntg, _q) in enumerate(GROUPS):
                dt = F16 if GROUP_F16[g] else F8
                t = xpool.tile([P, ntg * NTAIL], dt, name=f"xg{g}", tag=f"xg{g}")
                xg.append(t)
            x9n = sing.tile([P, NT, K], F32)      # x[:, 0:9] natural order
            pct = sing.tile([P, PC_W], F32)
            # scalar queue: ACT tiles + x9; sync queue: DVE singles + pc
            nc.scalar.dma_start(xg[0][:], xgs[0][:])      # t0
            nc.sync.dma_start(xg[3][:], xgs[3][:])        # t1
            nc.scalar.dma_start(xg[1][:], xgs[1][:])      # t2-3
            for g in (4, 5, 6, 7):                        # t4..t7
                nc.sync.dma_start(xg[g][:], xgs[g][:])
            nc.scalar.dma_start(x9n[:], x9_view)
            nc.scalar.dma_start(xg[2][:], xgs[2][:])      # t8-11
            for g in (8, 9, 10, 11):                      # t12..t15
                nc.sync.dma_start(xg[g][:], xgs[g][:])
            nc.sync.dma_start(pct[:], pc[:])

            # persistent small tiles
            d0s = sing.tile([P, NT, NO], F32)     # scan0 multiplier pattern
            d1s = sing.tile([P, NT, NO], F32)     # scan1 data (slot0=tail acc)
            s9a = sing.tile([P, NT], F32)         # ACT-tile accums
            s9v = sing.tile([P, NT], F32)         # DVE-tile accums
            scnb = sing.tile([P, NT, NO], F32)    # [0, sin_0..sin_8] per block
            d1p = sing.tile([P, NT, NO], F32)     # cumprod scan data1
            sqa = sing.tile([P, NTAIL], F32)      # ACT squares scratch (dead)
            sqd = sing.tile([P, NTAIL], F32)      # DVE squares scratch (dead)

            nc.gpsimd.memset(d0s[:], 1.0)
            nc.gpsimd.memset(d0s[:, :, 0:1], 0.0)
            nc.gpsimd.memset(s9a[:], 0.0)
            nc.gpsimd.memset(s9v[:], 0.0)
            nc.gpsimd.memset(scnb[:, :, 0:1], 0.0)
            nc.gpsimd.memset(d1p[:, :, 1:], 0.0)

            # ---- main loop: one engine per tile, accum -> d1s[:, t, 0] ----
            t2g = {}
            for g, (t0g, ntg, _q) in enumerate(GROUPS):
                for j in range(ntg):
                    t2g[t0g + j] = (g, j)
            for t in range(NT):
                g, j = t2g[t]
                src = xg[g][:, j * NTAIL:(j + 1) * NTAIL]
                if TILE_ENG[t] == 'A':
                    nc.scalar.activation(out=sqa[:], in_=src, func=AF.Square,
                                         accum_out=s9a[:, t:t + 1])
                elif DVE_USE_TTR:
                    nc.vector.tensor_tensor_reduce(
                        out=sqd[:], in0=src, in1=src, scale=1.0, scalar=0.0,
                        op0=OP.mult, op1=OP.add, accum_out=s9v[:, t:t + 1])
                else:
                    nc.vector.scalar_tensor_tensor(
                        out=sqd[:], in0=src, scalar=1.0, in1=src,
                        op0=OP.mult, op1=OP.mult, accum_out=s9v[:, t:t + 1])
            nc.vector.tensor_add(d1s[:, :, 0:1], s9a[:].unsqueeze(2),
                                 s9v[:].unsqueeze(2))
            # head squares into scan slots 1..9 (reversed order)
            nc.vector.tensor_mul(d1s[:, :, 1:NO], x9n[:, :, ::-1],
                                 x9n[:, :, ::-1])

            # ---- epilogue (all 16 row-tiles wide) ----
            ep = sing

            # suffix-sum scan: S[:, :, m] = s_{9-m} (m=1..9), S[:,:,0]=s_9
            S = ep.tile([P, NT, NO], F32)
            nc.vector.tensor_tensor_scan(
                out=S[:].rearrange("p b k -> p (b k)"),
                data0=d0s[:].rearrange("p b k -> p (b k)"),
                data1=d1s[:].rearrange("p b k -> p (b k)"),
                initial=0.0, op0=OP.mult, op1=OP.add,
            )

            # rsqrt: Quake seed + 1 Newton step (err ~4.6e-6)
            sbits = S[:, :, 1:NO].bitcast(I32)
            y0i = ep.tile([P, NT, K], I32)
            nc.vector.tensor_scalar(out=y0i[:], in0=sbits, scalar1=1,
                                    scalar2=-1, op0=OP.arith_shift_right,
                                    op1=OP.bitwise_xor)
            nc.vector.tensor_scalar(out=y0i[:], in0=y0i[:],
                                    scalar1=RSQRT_MAGIC + 1, scalar2=None,
                                    op0=OP.add)
            yv = y0i[:].bitcast(F32)
            # theta slots use the raw Quake seed (rel err <= 1.75e-3 ->
            # theta err ~3e-4 -> softmax norm err ~2e-4, fine); only the
            # r slot (multiplies all logits) gets a Newton step, on [P,NT,1]
            aa8 = ep.tile([P, NT, 1], F32)
            inv8 = ep.tile([P, NT, 1], F32)
            dacc = ep.tile([P, 1], F32)
            nc.vector.tensor_mul(aa8[:], yv[:, :, K - 1:K], yv[:, :, K - 1:K])
            nc.vector.tensor_mul(aa8[:], aa8[:], S[:, :, NO - 1:NO])
            nc.vector.affine_mul_reduce(out=inv8[:], accum_out=dacc[:],
                                        in0=aa8[:], in1=yv[:, :, K - 1:K],
                                        scale=-0.5, bias=1.5)

            # cumprod seed: r3 = c * s_0 * rsqrt(s_0)
            nc.vector.scalar_tensor_tensor(
                out=d1p[:, :, 0:1], in0=S[:, :, NO - 1:NO],
                scalar=pct[:, PC_C:PC_C + 1], in1=inv8[:],
                op0=OP.mult, op1=OP.mult,
            )

            # z = x * seed-rsqrt(s) (reversed order), arcsin series (2 terms)
            z = ep.tile([P, NT, K], F32)
            nc.vector.tensor_mul(z[:], x9n[:, :, ::-1], yv)
            u2 = ep.tile([P, NT, K], F32)
            nc.vector.tensor_mul(u2[:], z[:], z[:])
            w2 = ep.tile([P, NT, K], F32)
            nc.vector.affine_mul_reduce(out=w2[:], accum_out=dacc[:],
                                        in0=u2[:], in1=u2[:], scale=C5,
                                        bias=C3)
            asin = ep.tile([P, NT, K], F32)
            nc.vector.scalar_tensor_tensor(out=asin[:], in0=w2[:], scalar=1.0,
                                           op0=OP.add, op1=OP.mult, in1=z[:])
            # asin = (1 + C3 u2 + C5 u2^2) * z

            def bc(col):  # broadcast pc[col:col+9] over the NT dim
                return (pct[:, col:col + K].unsqueeze(1)
                        .broadcast_to([P, NT, K]))

            # th = max(max(c123 - asin, r23), rot3)
            th = ep.tile([P, NT, K], F32)
            nc.vector.scalar_tensor_tensor(out=th[:], in0=asin[:], scalar=-1.0,
                                           in1=bc(PC_C123), op0=OP.mult,
                                           op1=OP.add)
            nc.vector.tensor_tensor(out=th[:], in0=th[:], in1=bc(PC_R23),
                                    op=OP.max)
            nc.vector.tensor_tensor(out=th[:], in0=th[:], in1=bc(PC_R3),
                                    op=OP.max)

            if range_reduce:
                # range-reduce into [-pi, pi]
                un = ep.tile([P, NT, K], F32)
                nv = ep.tile([P, NT, K], F32)
                nc.vector.tensor_scalar(out=un[:], in0=th[:], scalar1=INV_2PI,
                                        scalar2=MAGIC, op0=OP.mult, op1=OP.add)
                nc.vector.tensor_scalar(out=nv[:], in0=un[:], scalar1=MAGIC,
                                        scalar2=None, op0=OP.subtract)
                thpT = ep.tile([P, NT, K], F32)
                nc.vector.scalar_tensor_tensor(out=thpT[:], in0=nv[:],
                                               scalar=-TWO_PI, in1=th[:],
                                               op0=OP.mult, op1=OP.add)
                thp = thpT[:]
            else:
                thp = th[:]

            # sins (natural order) into scnb slots 1..9; cos via half-angle
            # (the post-sin DVE ops overlap the exp table load on ACT)
            nc.scalar.activation(scnb[:, :, 1:NO], thp[:, :, ::-1], AF.Sin)
            sh = ep.tile([P, NT, K], F32)
            nc.scalar.activation(sh[:], thp, AF.Sin, scale=0.5)
            ccr = ep.tile([P, NT, K], F32)
            nc.vector.tensor_mul(ccr[:], sh[:], sh[:])
            nc.vector.tensor_scalar(out=ccr[:], in0=ccr[:], scalar1=-2.0,
                                    scalar2=1.0, op0=OP.mult, op1=OP.add)

            # cumprod scan: PP[:, :, m] = r3 * prod_{i<m} sin_i
            PP = ep.tile([P, NT, NO], F32)
            nc.vector.tensor_tensor_scan(
                out=PP[:].rearrange("p b k -> p (b k)"),
                data0=scnb[:].rearrange("p b k -> p (b k)"),
                data1=d1p[:].rearrange("p b k -> p (b k)"),
                initial=0.0, op0=OP.mult, op1=OP.add,
            )

            lg = ep.tile([P, NT, NO], F32)
            nc.vector.tensor_mul(lg[:, :, 0:1], PP[:, :, K - 1:K],
                                 ccr[:, :, 0:1])
            nc.vector.tensor_mul(lg[:, :, 1:2], PP[:, :, K - 1:K],
                                 scnb[:, :, NO - 1:NO])
            nc.vector.tensor_mul(lg[:, :, 2:NO], PP[:, :, 7::-1],
                                 ccr[:, :, 1:K])

            # softmax without max-sub (|logits| <= ~45, f32-safe),
            # pipelined in halves so the first output DMA issues early
            E = ep.tile([P, NT, NO], F32)
            ds = ep.tile([P, NT], F32)
            dinv = ep.tile([P, NT], F32)
            out = ep.tile([P, NT, NO], F32)
            H = NT // 2
            for h, (lo, hi) in enumerate(((0, H), (H, NT))):
                w = hi - lo
                nc.scalar.activation(E[:, lo:hi, :], lg[:, lo:hi, :], AF.Exp)
                nc.vector.tensor_reduce(out=ds[:, lo:hi], in_=E[:, lo:hi, :],
                                        axis=mybir.AxisListType.X, op=OP.add)
                nc.vector.reciprocal(dinv[:, lo:hi], ds[:, lo:hi])
                nc.vector.tensor_mul(
                    out[:, lo:hi, :], E[:, lo:hi, :],
                    dinv[:, lo:hi].unsqueeze(2).broadcast_to([P, w, NO]))
                eng = nc.sync if h == 0 else nc.scalar
                eng.dma_start(y_view[:, lo:hi, :], out[:, lo:hi, :])

    nc.compile()
    return nc


_NC = None
_NC_RR = None


def _get_nc(range_reduce):
    global _NC, _NC_RR
    if range_reduce:
        if _NC_RR is None:
            _NC_RR = _build(True)
        return _NC_RR
    if _NC is None:
        _NC = _build(False)
    return _NC


def _host_params(scale1, rot1, scale2, rot2, scale3, rot3):
    c = max(max(float(scale1[0]), 0.0) * float(scale2[0]), 0.0) * float(scale3[0])
    rev = np.arange(8, -1, -1)
    r1 = rot1[:K].astype(np.float64)[rev]
    r2 = rot2[:K].astype(np.float64)[rev]
    r3 = rot3[:K].astype(np.float64)[rev]
    c123 = np.pi / 2 + r1 + r2 + r3
    r23 = r2 + r3
    # per-slot 2*pi*n shift: th = max(c123 - asin, r23, r3) is invariant
    # under shifting all three constants by -2*pi*n (sin period); choose n
    # so the reachable th range lands within [-HOST_SHIFT_LIMIT, +...]
    A = 0.35                               # |asin| bound (actual max ~0.19)
    th_min = np.maximum.reduce([c123 - A, r23, r3])
    th_max = np.maximum.reduce([c123 + A, r23, r3])
    n = np.round((th_min + th_max) / 2 / (2 * np.pi))
    range_reduce = bool(
        (th_max - 2 * np.pi * n > HOST_SHIFT_LIMIT).any()
        or (th_min - 2 * np.pi * n < -HOST_SHIFT_LIMIT).any())
    if not range_reduce:
        c123 = c123 - 2 * np.pi * n
        r23 = r23 - 2 * np.pi * n
        r3 = r3 - 2 * np.pi * n
    row = np.zeros((PC_W,), np.float64)
    row[PC_C] = c
    row[PC_C123:PC_C123 + K] = c123
    row[PC_R23:PC_R23 + K] = r23
    row[PC_R3:PC_R3 + K] = r3
    row[PC_HPI] = np.pi / 2
    return np.tile(row.astype(np.float32)[None, :], (P, 1)), range_reduce


def kernel(x, scale1, rot1, scale2, rot2, scale3, rot3, _trace=False):
    pc, range_reduce = _host_params(scale1, rot1, scale2, rot2, scale3, rot3)
    nc = _get_nc(range_reduce)
    x = np.ascontiguousarray(x, dtype=np.float32)
    x9h = np.ascontiguousarray(x[:, 0:K])
    tail = x[:, K:]                                     # [B, 775]
    t16 = tail.astype(np.float16)
    t8 = tail.astype(ml_dtypes.float8_e4m3fn)
    in_maps = []
    for cidx in range(NCORES):
        m = {"pc": pc, "x9": x9h[cidx * ROWS:(cidx + 1) * ROWS]}
        for g, (t0g, ntg, _q) in enumerate(GROUPS):
            src = t16 if GROUP_F16[g] else t8
            # partition p, slot j -> row 16*p + t0g + j
            blk = src[cidx * ROWS:(cidx + 1) * ROWS].reshape(P, NT, NTAIL)
            m[f"xg{g}"] = np.ascontiguousarray(
                blk[:, t0g:t0g + ntg, :]).reshape(P, ntg * NTAIL)
        in_maps.append(m)
    res = run_bass_kernel_spmd(nc, in_maps, core_ids=list(range(NCORES)),
                               trace=_trace)
    outp = np.concatenate([res.results[c]["y"] for c in range(NCORES)], axis=0)
    if _trace:
        return outp, res
    return outp


# revision 13
# speedup vs baseline: 1.0933x; 1.0242x over previous
"""Trainium2 Bass kernel for nn_Net_2491081031714 — v2.

Math (per row x of 784 f32):
  s_k = sum_{j>=k} x_j^2 (k=0..8), r = sqrt(s_0)
  theta_k = arccos(x_k / sqrt(s_k))  (k=0..8)
  th_k = relu(relu(theta_k + rot1_k) + rot2_k) + rot3_k
      = max(max(c123_k - asin(z_k), r23_k), rot3_k),  c123 = pi/2+rot1+rot2+rot3
  r3 = r * c,  c = relu(relu(scale1)*scale2)*scale3
  cart = polar_to_cartesian(r3, th); out = softmax(cart)

v2 changes vs v1:
  - tail columns (9..784) shipped as fp8 e4m3 (half the DMA bytes);
    head columns x[:,0:9] stay f32 (theta numerators need precision)
  - main loop: whole-tile engine assignment (ACT Square+accum vs DVE
    STT/TTR+accum) instead of per-tile column split — each tile pays one
    engine's fixed cost instead of two
  - tail accums land directly in the scan input tile (no gather pass)
  - epilogue: fused relu chain (3 ops), 1 Newton rsqrt step (seed err
    1.75e-3 -> 4.6e-6), 4-op arcsin series, rot constants broadcast from
    [P,9] (pc shrinks 377KB -> 14KB)
  - scan d0 patterns built with memsets instead of DMA'd

Sharding: pure batch data-parallel over 8 cores (2048 rows each).
"""

import numpy as np
import ml_dtypes

import concourse.bacc as bacc
import concourse.tile as tile
from concourse import mybir
from concourse.bass_utils import run_bass_kernel_spmd

AF = mybir.ActivationFunctionType
OP = mybir.AluOpType
F32 = mybir.dt.float32
I32 = mybir.dt.int32
F16 = mybir.dt.float16
F8 = mybir.dt.float8e4

B, N = 16384, 784
NCORES = 8
ROWS = B // NCORES          # 2048
P = 128
NT = ROWS // P              # 16 row-tiles per core
# input DMA groups: (first tile, #tiles, queue). Each engine is fed from
# its own HWDGE queue so arrivals pace consumption independently:
# ACT tiles {0,2,3,8-11} via scalar-queue groups, DVE tiles as singles
# via the sync queue (issue rate 0.62us/tile < consumption 0.97us/tile).
GROUPS = [(0, 1, 'a'), (2, 2, 'a'), (8, 4, 'a'),
          (1, 1, 's'), (4, 1, 's'), (5, 1, 's'), (6, 1, 's'), (7, 1, 's'),
          (12, 1, 's'), (13, 1, 's'), (14, 1, 's'), (15, 1, 's')]
NG = len(GROUPS)
K = 9                       # thetas that matter
NO = 10                     # output classes
NTAIL = N - K               # 775 tail columns

TWO_PI = 6.283185307179586
INV_2PI = 1.0 / TWO_PI
MAGIC = 1.5 * 2 ** 23       # round-to-nearest trick for |u| << 2^22
RSQRT_MAGIC = 0x5F3759DF    # Quake rsqrt seed constant
C3 = 1.0 / 6.0              # arcsin series z^3 coeff
C5 = 3.0 / 40.0             # arcsin series z^5 coeff

# ---- tuning knobs -------------------------------------------------------
# dtype per DMA group (2 tiles each): True -> f16, False -> fp8
GROUP_F16 = [False] * 12
# engine per tile: 'A' = ACT Square+accum, 'V' = DVE
TILE_ENG = ['A', 'V', 'A', 'A', 'V', 'V', 'V', 'V',
            'A', 'A', 'A', 'A', 'V', 'V', 'V', 'V']  # 7 ACT / 9 DVE
DVE_USE_TTR = False         # tensor_tensor_reduce (crashes TRN2 exec unit!)
# range handling: host folds a per-slot 2*pi*n shift into c123/r23/rot3 so
# th lands in [-pi, pi] (verified against Sin table accuracy up to |3.0|);
# if the actual rotations don't allow it, fall back to on-device reduction
HOST_SHIFT_LIMIT = 3.0
# ------------------------------------------------------------------------

# pc (host-prepared params) column layout
PC_C = 0                    # scale product c
PC_C123 = 1                 # pi/2 + rot1 + rot2 + rot3, reversed [9]
PC_R23 = PC_C123 + K        # rot2 + rot3, reversed [9]
PC_R3 = PC_R23 + K          # rot3, reversed [9]
PC_HPI = PC_R3 + K          # pi/2 constant (activation bias needs an AP)
PC_W = PC_HPI + 1


def _build(range_reduce):
    nc = bacc.Bacc("TRN2", target_bir_lowering=False, debug=False)
    xgs = []
    for g, (t0g, ntg, _q) in enumerate(GROUPS):
        dt = F16 if GROUP_F16[g] else F8
        xgs.append(nc.dram_tensor(f"xg{g}", [P, ntg * NTAIL], dt,
                                  kind="ExternalInput"))
    x9 = nc.dram_tensor("x9", [ROWS, K], F32, kind="ExternalInput")
    pc = nc.dram_tensor("pc", [P, PC_W], F32, kind="ExternalInput")
    y = nc.dram_tensor("y", [ROWS, NO], F32, kind="ExternalOutput")

    # row <-> (partition, slot): row = 16*p + t
    x9_view = x9.rearrange("(p t) k -> p t k", p=P)             # [P, NT, K]
    y_view = y.rearrange("(p t) k -> p t k", p=P)               # [P, NT, NO]

    with tile.TileContext(nc) as tc:
        with (
            tc.tile_pool(name="xpool", bufs=1) as xpool,
            tc.tile_pool(name="sing", bufs=1) as sing,
        ):
            # ACT table preload: first ACTIVATE being Sin pulls in
            # trig_and_small (sin+square+relu) under the DMA ramp.
            warm = sing.tile([P, 1], F32)
            nc.vector.memset(warm[:], 0.0)
            nc.scalar.activation(warm[:], warm[:], AF.Sin)

            # input DMAs: tail groups first (alternate sync/scalar issue
            # queues), then the small epilogue tensors
            xg = []
            for g, (t0g, ntg, _q) in enumerate(GROUPS):
                dt = F16 if GROUP_F16[g] else F8
                t = xpool.tile([P, ntg * NTAIL], dt, name=f"xg{g}", tag=f"xg{g}")
                xg.append(t)
            x9n = sing.tile([P, NT, K], F32)      # x[:, 0:9] natural order
            pct = sing.tile([P, PC_W], F32)
            # scalar queue: ACT tiles + x9; sync queue: DVE singles + pc
            nc.scalar.dma_start(xg[0][:], xgs[0][:])      # t0
            nc.sync.dma_start(xg[3][:], xgs[3][:])        # t1
            nc.scalar.dma_start(xg[1][:], xgs[1][:])      # t2-3
            for g in (4, 5, 6, 7):                        # t4..t7
                nc.sync.dma_start(xg[g][:], xgs[g][:])
            nc.scalar.dma_start(x9n[:], x9_view)
            nc.scalar.dma_start(xg[2][:], xgs[2][:])      # t8-11
            for g in (8, 9, 10, 11):                      # t12..t15
                nc.sync.dma_start(xg[g][:], xgs[g][:])
            nc.sync.dma_start(pct[:], pc[:])

            # persistent small tiles
            d0s = sing.tile([P, NT, NO], F32)     # scan0 multiplier pattern
            d1s = sing.tile([P, NT, NO], F32)     # scan1 data (slot0=tail acc)
            s9a = sing.tile([P, NT], F32)         # ACT-tile accums
            s9v = sing.tile([P, NT], F32)         # DVE-tile accums
            scnb = sing.tile([P, NT, NO], F32)    # [0, sin_0..sin_8] per block
            d1p = sing.tile([P, NT, NO], F32)     # cumprod scan data1
            sqa = sing.tile([P, NTAIL], F32)      # ACT squares scratch (dead)
            sqd = sing.tile([P, NTAIL], F32)      # DVE squares scratch (dead)

            nc.gpsimd.memset(d0s[:], 1.0)
            nc.gpsimd.memset(d0s[:, :, 0:1], 0.0)
            nc.gpsimd.memset(s9a[:], 0.0)
            nc.gpsimd.memset(s9v[:], 0.0)
            nc.gpsimd.memset(scnb[:, :, 0:1], 0.0)
            nc.gpsimd.memset(d1p[:, :, 1:], 0.0)

            # ---- main loop: one engine per tile, accum -> d1s[:, t, 0] ----
            t2g = {}
            for g, (t0g, ntg, _q) in enumerate(GROUPS):
                for j in range(ntg):
                    t2g[t0g + j] = (g, j)
            for t in range(NT):
                g, j = t2g[t]
                src = xg[g][:, j * NTAIL:(j + 1) * NTAIL]
                if TILE_ENG[t] == 'A':
                    nc.scalar.activation(out=sqa[:], in_=src, func=AF.Square,
                                         accum_out=d1s[:, t, 0:1])
                else:
                    nc.vector.scalar_tensor_tensor(
                        out=sqd[:], in0=src, scalar=1.0, in1=src,
                        op0=OP.mult, op1=OP.mult, accum_out=d1s[:, t, 0:1])
            # head squares into scan slots 1..9 (reversed order)
            nc.vector.tensor_mul(d1s[:, :, 1:NO], x9n[:, :, ::-1],
                                 x9n[:, :, ::-1])

            # ---- epilogue (all 16 row-tiles wide) ----
            ep = sing

            # suffix-sum scan: S[:, :, m] = s_{9-m} (m=1..9), S[:,:,0]=s_9
            S = ep.tile([P, NT, NO], F32)
            nc.vector.tensor_tensor_scan(
                out=S[:].rearrange("p b k -> p (b k)"),
                data0=d0s[:].rearrange("p b k -> p (b k)"),
                data1=d1s[:].rearrange("p b k -> p (b k)"),
                initial=0.0, op0=OP.mult, op1=OP.add,
            )

            # rsqrt: Quake seed + 1 Newton step (err ~4.6e-6)
            sbits = S[:, :, 1:NO].bitcast(I32)
            y0i = ep.tile([P, NT, K], I32)
            nc.vector.tensor_scalar(out=y0i[:], in0=sbits, scalar1=1,
                                    scalar2=-1, op0=OP.arith_shift_right,
                                    op1=OP.bitwise_xor)
            nc.vector.tensor_scalar(out=y0i[:], in0=y0i[:],
                                    scalar1=RSQRT_MAGIC + 1, scalar2=None,
                                    op0=OP.add)
            yv = y0i[:].bitcast(F32)
            # theta slots use the raw Quake seed (rel err <= 1.75e-3 ->
            # theta err ~3e-4 -> softmax norm err ~2e-4, fine); only the
            # r slot (multiplies all logits) gets a Newton step, on [P,NT,1]
            aa8 = ep.tile([P, NT, 1], F32)
            inv8 = ep.tile([P, NT, 1], F32)
            dacc = ep.tile([P, 1], F32)
            nc.vector.tensor_mul(aa8[:], yv[:, :, K - 1:K], yv[:, :, K - 1:K])
            nc.vector.tensor_mul(aa8[:], aa8[:], S[:, :, NO - 1:NO])
            nc.vector.affine_mul_reduce(out=inv8[:], accum_out=dacc[:],
                                        in0=aa8[:], in1=yv[:, :, K - 1:K],
                                        scale=-0.5, bias=1.5)

            # cumprod seed: r3 = c * s_0 * rsqrt(s_0)
            nc.vector.scalar_tensor_tensor(
                out=d1p[:, :, 0:1], in0=S[:, :, NO - 1:NO],
                scalar=pct[:, PC_C:PC_C + 1], in1=inv8[:],
                op0=OP.mult, op1=OP.mult,
            )

            # z = x * seed-rsqrt(s) (reversed order), arcsin series (2 terms)
            z = ep.tile([P, NT, K], F32)
            nc.vector.tensor_mul(z[:], x9n[:, :, ::-1], yv)
            u2 = ep.tile([P, NT, K], F32)
            nc.vector.tensor_mul(u2[:], z[:], z[:])
            w2 = ep.tile([P, NT, K], F32)
            nc.vector.affine_mul_reduce(out=w2[:], accum_out=dacc[:],
                                        in0=u2[:], in1=u2[:], scale=C5,
                                        bias=C3)
            asin = ep.tile([P, NT, K], F32)
            nc.vector.scalar_tensor_tensor(out=asin[:], in0=w2[:], scalar=1.0,
                                           op0=OP.add, op1=OP.mult, in1=z[:])
            # asin = (1 + C3 u2 + C5 u2^2) * z

            def bc(col):  # broadcast pc[col:col+9] over the NT dim
                return (pct[:, col:col + K].unsqueeze(1)
                        .broadcast_to([P, NT, K]))

            # th = max(max(c123 - asin, r23), rot3)
            th = ep.tile([P, NT, K], F32)
            nc.vector.scalar_tensor_tensor(out=th[:], in0=asin[:], scalar=-1.0,
                                           in1=bc(PC_C123), op0=OP.mult,
                                           op1=OP.add)
            nc.vector.tensor_tensor(out=th[:], in0=th[:], in1=bc(PC_R23),
                                    op=OP.max)
            nc.vector.tensor_tensor(out=th[:], in0=th[:], in1=bc(PC_R3),
                                    op=OP.max)

            if range_reduce:
                # range-reduce into [-pi, pi]
                un = ep.tile([P, NT, K], F32)
                nv = ep.tile([P, NT, K], F32)
                nc.vector.tensor_scalar(out=un[:], in0=th[:], scalar1=INV_2PI,
                                        scalar2=MAGIC, op0=OP.mult, op1=OP.add)
                nc.vector.tensor_scalar(out=nv[:], in0=un[:], scalar1=MAGIC,
                                        scalar2=None, op0=OP.subtract)
                thpT = ep.tile([P, NT, K], F32)
                nc.vector.scalar_tensor_tensor(out=thpT[:], in0=nv[:],
                                               scalar=-TWO_PI, in1=th[:],
                                               op0=OP.mult, op1=OP.add)
                thp = thpT[:]
            else:
                thp = th[:]

            # sins (natural order) into scnb slots 1..9; cos via half-angle
            # (the post-sin DVE ops overlap the exp table load on ACT)
            nc.scalar.activation(scnb[:, :, 1:NO], thp[:, :, ::-1], AF.Sin)
            sh = ep.tile([P, NT, K], F32)
            nc.scalar.activation(sh[:], thp, AF.Sin, scale=0.5)
            ccr = ep.tile([P, NT, K], F32)
            nc.vector.tensor_mul(ccr[:], sh[:], sh[:])
            nc.vector.tensor_scalar(out=ccr[:], in0=ccr[:], scalar1=-2.0,
                                    scalar2=1.0, op0=OP.mult, op1=OP.add)

            # cumprod scan: PP[:, :, m] = r3 * prod_{i<m} sin_i
            PP = ep.tile([P, NT, NO], F32)
            nc.vector.tensor_tensor_scan(
                out=PP[:].rearrange("p b k -> p (b k)"),
                data0=scnb[:].rearrange("p b k -> p (b k)"),
                data1=d1p[:].rearrange("p b k -> p (b k)"),
                initial=0.0, op0=OP.mult, op1=OP.add,
            )

            lg = ep.tile([P, NT, NO], F32)
            nc.vector.tensor_mul(lg[:, :, 0:1], PP[:, :, K - 1:K],
                                 ccr[:, :, 0:1])
            nc.vector.tensor_mul(lg[:, :, 1:2], PP[:, :, K - 1:K],
                                 scnb[:, :, NO - 1:NO])
            nc.vector.tensor_mul(lg[:, :, 2:NO], PP[:, :, 7::-1],
                                 ccr[:, :, 1:K])

            # softmax without max-sub (|logits| <= ~45, f32-safe),
            # pipelined in halves so the first output DMA issues early
            E = ep.tile([P, NT, NO], F32)
            ds = ep.tile([P, NT], F32)
            dinv = ep.tile([P, NT], F32)
            out = ep.tile([P, NT, NO], F32)
            H = NT // 2
            for h, (lo, hi) in enumerate(((0, H), (H, NT))):
                w = hi - lo
                nc.scalar.activation(E[:, lo:hi, :], lg[:, lo:hi, :], AF.Exp)
                nc.vector.tensor_reduce(out=ds[:, lo:hi], in_=E[:, lo:hi, :],
                                        axis=mybir.AxisListType.X, op=OP.add)
                nc.vector.reciprocal(dinv[:, lo:hi], ds[:, lo:hi])
                nc.vector.tensor_mul(
                    out[:, lo:hi, :], E[:, lo:hi, :],
                    dinv[:, lo:hi].unsqueeze(2).broadcast_to([P, w, NO]))
                eng = nc.sync if h == 0 else nc.scalar
                eng.dma_start(y_view[:, lo:hi, :], out[:, lo:hi, :])

    nc.compile()
    return nc


_NC = None
_NC_RR = None


def _get_nc(range_reduce):
    global _NC, _NC_RR
    if range_reduce:
        if _NC_RR is None:
            _NC_RR = _build(True)
        return _NC_RR
    if _NC is None:
        _NC = _build(False)
    return _NC


def _host_params(scale1, rot1, scale2, rot2, scale3, rot3):
    c = max(max(float(scale1[0]), 0.0) * float(scale2[0]), 0.0) * float(scale3[0])
    rev = np.arange(8, -1, -1)
    r1 = rot1[:K].astype(np.float64)[rev]
    r2 = rot2[:K].astype(np.float64)[rev]
    r3 = rot3[:K].astype(np.float64)[rev]
    c123 = np.pi / 2 + r1 + r2 + r3
    r23 = r2 + r3
    # per-slot 2*pi*n shift: th = max(c123 - asin, r23, r3) is invariant
    # under shifting all three constants by -2*pi*n (sin period); choose n
    # so the reachable th range lands within [-HOST_SHIFT_LIMIT, +...]
    A = 0.35                               # |asin| bound (actual max ~0.19)
    th_min = np.maximum.reduce([c123 - A, r23, r3])
    th_max = np.maximum.reduce([c123 + A, r23, r3])
    n = np.round((th_min + th_max) / 2 / (2 * np.pi))
    range_reduce = bool(
        (th_max - 2 * np.pi * n > HOST_SHIFT_LIMIT).any()
        or (th_min - 2 * np.pi * n < -HOST_SHIFT_LIMIT).any())
    if not range_reduce:
        c123 = c123 - 2 * np.pi * n
        r23 = r23 - 2 * np.pi * n
        r3 = r3 - 2 * np.pi * n
    row = np.zeros((PC_W,), np.float64)
    row[PC_C] = c
    row[PC_C123:PC_C123 + K] = c123
    row[PC_R23:PC_R23 + K] = r23
    row[PC_R3:PC_R3 + K] = r3
    row[PC_HPI] = np.pi / 2
    return np.tile(row.astype(np.float32)[None, :], (P, 1)), range_reduce


def kernel(x, scale1, rot1, scale2, rot2, scale3, rot3, _trace=False):
    pc, range_reduce = _host_params(scale1, rot1, scale2, rot2, scale3, rot3)
    nc = _get_nc(range_reduce)
    x = np.ascontiguousarray(x, dtype=np.float32)
    x9h = np.ascontiguousarray(x[:, 0:K])
    tail = x[:, K:]                                     # [B, 775]
    t16 = tail.astype(np.float16)
    t8 = tail.astype(ml_dtypes.float8_e4m3fn)
    in_maps = []
    for cidx in range(NCORES):
        m = {"pc": pc, "x9": x9h[cidx * ROWS:(cidx + 1) * ROWS]}
        for g, (t0g, ntg, _q) in enumerate(GROUPS):
            src = t16 if GROUP_F16[g] else t8
            # partition p, slot j -> row 16*p + t0g + j
            blk = src[cidx * ROWS:(cidx + 1) * ROWS].reshape(P, NT, NTAIL)
            m[f"xg{g}"] = np.ascontiguousarray(
                blk[:, t0g:t0g + ntg, :]).reshape(P, ntg * NTAIL)
        in_maps.append(m)
    res = run_bass_kernel_spmd(nc, in_maps, core_ids=list(range(NCORES)),
                               trace=_trace)
    outp = np.concatenate([res.results[c]["y"] for c in range(NCORES)], axis=0)
    if _trace:
        return outp, res
    return outp


# revision 14
# speedup vs baseline: 1.0985x; 1.0048x over previous
"""Trainium2 Bass kernel for nn_Net_2491081031714 — v2.

Math (per row x of 784 f32):
  s_k = sum_{j>=k} x_j^2 (k=0..8), r = sqrt(s_0)
  theta_k = arccos(x_k / sqrt(s_k))  (k=0..8)
  th_k = relu(relu(theta_k + rot1_k) + rot2_k) + rot3_k
      = max(max(c123_k - asin(z_k), r23_k), rot3_k),  c123 = pi/2+rot1+rot2+rot3
  r3 = r * c,  c = relu(relu(scale1)*scale2)*scale3
  cart = polar_to_cartesian(r3, th); out = softmax(cart)

Design (measured 31.1us vs 40.5us baseline on TRN2, rel err 2.1e-4):
  - tail columns (9..784) shipped as fp8 e4m3 (half the DMA bytes);
    head columns x[:,0:9] stay f32 (theta numerators need precision)
  - main loop: whole-tile engine assignment (7 tiles ACT Square+accum,
    9 tiles DVE STT+accum) — each tile pays one engine's fixed cost;
    accums land directly in the scan-input tile slot 0
  - per-engine DMA feeding: DVE tiles as per-tile DMAs on the sync
    HWDGE queue, ACT tiles as [1,2,4]-tile groups on the scalar queue,
    so each engine's arrivals pace its own consumption
  - epilogue: suffix-sum scan; theta z-values use the raw Quake rsqrt
    seed (err 1.75e-3 -> softmax norm err ~2e-4), only the r slot gets
    a Newton step; fused relu chain (3 ops); host folds per-slot 2*pi*n
    shifts into the rotation constants so Sin stays in its accurate
    domain (fallback: on-device range reduction); cos via half-angle
    (its DVE ops hide the exp ACT-table load); softmax in halves so
    output DMA starts early on both queues

Sharding: pure batch data-parallel over 8 cores (2048 rows each).
"""

import numpy as np
import ml_dtypes

import concourse.bacc as bacc
import concourse.tile as tile
from concourse import mybir
from concourse.bass_utils import run_bass_kernel_spmd

AF = mybir.ActivationFunctionType
OP = mybir.AluOpType
F32 = mybir.dt.float32
I32 = mybir.dt.int32
F16 = mybir.dt.float16
F8 = mybir.dt.float8e4

B, N = 16384, 784
NCORES = 8
ROWS = B // NCORES          # 2048
P = 128
NT = ROWS // P              # 16 row-tiles per core
# input DMA groups: (first tile, #tiles, queue). Each engine is fed from
# its own HWDGE queue so arrivals pace consumption independently:
# ACT tiles {0,2,3,8-11} via scalar-queue groups, DVE tiles as singles
# via the sync queue (issue rate 0.62us/tile < consumption 0.97us/tile).
GROUPS = [(0, 1, 'a'), (2, 2, 'a'), (8, 4, 'a'),
          (1, 1, 's'), (4, 1, 's'), (5, 1, 's'), (6, 1, 's'), (7, 1, 's'),
          (12, 1, 's'), (13, 1, 's'), (14, 1, 's'), (15, 1, 's')]
NG = len(GROUPS)
K = 9                       # thetas that matter
NO = 10                     # output classes
NTAIL = N - K               # 775 tail columns

TWO_PI = 6.283185307179586
INV_2PI = 1.0 / TWO_PI
MAGIC = 1.5 * 2 ** 23       # round-to-nearest trick for |u| << 2^22
RSQRT_MAGIC = 0x5F3759DF    # Quake rsqrt seed constant
C3 = 1.0 / 6.0              # arcsin series z^3 coeff
C5 = 3.0 / 40.0             # arcsin series z^5 coeff

# ---- tuning knobs -------------------------------------------------------
# dtype per DMA group (2 tiles each): True -> f16, False -> fp8
GROUP_F16 = [False] * 12
# engine per tile: 'A' = ACT Square+accum, 'V' = DVE
TILE_ENG = ['A', 'V', 'A', 'A', 'V', 'V', 'V', 'V',
            'A', 'A', 'A', 'A', 'V', 'V', 'V', 'V']  # 7 ACT / 9 DVE
# range handling: host folds a per-slot 2*pi*n shift into c123/r23/rot3 so
# th lands in [-pi, pi] (verified against Sin table accuracy up to |3.0|);
# if the actual rotations don't allow it, fall back to on-device reduction
HOST_SHIFT_LIMIT = 3.0
# ------------------------------------------------------------------------

# pc (host-prepared params) column layout
PC_C = 0                    # scale product c
PC_C123 = 1                 # pi/2 + rot1 + rot2 + rot3, reversed [9]
PC_R23 = PC_C123 + K        # rot2 + rot3, reversed [9]
PC_R3 = PC_R23 + K          # rot3, reversed [9]
PC_HPI = PC_R3 + K          # pi/2 constant (activation bias needs an AP)
PC_W = PC_HPI + 1


def _build(range_reduce):
    nc = bacc.Bacc("TRN2", target_bir_lowering=False, debug=False)
    xgs = []
    for g, (t0g, ntg, _q) in enumerate(GROUPS):
        dt = F16 if GROUP_F16[g] else F8
        xgs.append(nc.dram_tensor(f"xg{g}", [P, ntg * NTAIL], dt,
                                  kind="ExternalInput"))
    x9 = nc.dram_tensor("x9", [ROWS, K], F32, kind="ExternalInput")
    pc = nc.dram_tensor("pc", [P, PC_W], F32, kind="ExternalInput")
    y = nc.dram_tensor("y", [ROWS, NO], F32, kind="ExternalOutput")

    # row <-> (partition, slot): row = 16*p + t
    x9_view = x9.rearrange("(p t) k -> p t k", p=P)             # [P, NT, K]
    y_view = y.rearrange("(p t) k -> p t k", p=P)               # [P, NT, NO]

    with tile.TileContext(nc) as tc:
        with (
            tc.tile_pool(name="xpool", bufs=1) as xpool,
            tc.tile_pool(name="sing", bufs=1) as sing,
        ):
            # ACT table preload: first ACTIVATE being Sin pulls in
            # trig_and_small (sin+square+relu) under the DMA ramp.
            warm = sing.tile([P, 1], F32)
            nc.vector.memset(warm[:], 0.0)
            nc.scalar.activation(warm[:], warm[:], AF.Sin)

            # input DMAs: tail groups first (alternate sync/scalar issue
            # queues), then the small epilogue tensors
            xg = []
            for g, (t0g, ntg, _q) in enumerate(GROUPS):
                dt = F16 if GROUP_F16[g] else F8
                t = xpool.tile([P, ntg * NTAIL], dt, name=f"xg{g}", tag=f"xg{g}")
                xg.append(t)
            x9n = sing.tile([P, NT, K], F32)      # x[:, 0:9] natural order
            pct = sing.tile([P, PC_W], F32)
            # scalar queue: ACT tiles + x9; sync queue: DVE singles + pc
            nc.scalar.dma_start(xg[0][:], xgs[0][:])      # t0
            nc.sync.dma_start(xg[3][:], xgs[3][:])        # t1
            nc.scalar.dma_start(xg[1][:], xgs[1][:])      # t2-3
            for g in (4, 5, 6, 7):                        # t4..t7
                nc.sync.dma_start(xg[g][:], xgs[g][:])
            nc.scalar.dma_start(x9n[:], x9_view)
            nc.scalar.dma_start(xg[2][:], xgs[2][:])      # t8-11
            for g in (8, 9, 10, 11):                      # t12..t15
                nc.sync.dma_start(xg[g][:], xgs[g][:])
            nc.sync.dma_start(pct[:], pc[:])

            # persistent small tiles
            d0s = sing.tile([P, NT, NO], F32)     # scan0 multiplier pattern
            d1s = sing.tile([P, NT, NO], F32)     # scan1 data (slot0=tail acc)
            scnb = sing.tile([P, NT, NO], F32)    # [0, sin_0..sin_8] per block
            d1p = sing.tile([P, NT, NO], F32)     # cumprod scan data1
            sqa = sing.tile([P, NTAIL], F32)      # ACT squares scratch (dead)
            sqd = sing.tile([P, NTAIL], F32)      # DVE squares scratch (dead)

            nc.gpsimd.memset(d0s[:], 1.0)
            nc.gpsimd.memset(d0s[:, :, 0:1], 0.0)
            nc.gpsimd.memset(scnb[:, :, 0:1], 0.0)
            nc.gpsimd.memset(d1p[:, :, 1:], 0.0)

            # ---- main loop: one engine per tile, accum -> d1s[:, t, 0] ----
            t2g = {}
            for g, (t0g, ntg, _q) in enumerate(GROUPS):
                for j in range(ntg):
                    t2g[t0g + j] = (g, j)
            for t in range(NT):
                g, j = t2g[t]
                src = xg[g][:, j * NTAIL:(j + 1) * NTAIL]
                if TILE_ENG[t] == 'A':
                    nc.scalar.activation(out=sqa[:], in_=src, func=AF.Square,
                                         accum_out=d1s[:, t, 0:1])
                else:
                    nc.vector.scalar_tensor_tensor(
                        out=sqd[:], in0=src, scalar=1.0, in1=src,
                        op0=OP.mult, op1=OP.mult, accum_out=d1s[:, t, 0:1])
            # head squares into scan slots 1..9 (reversed order)
            nc.vector.tensor_mul(d1s[:, :, 1:NO], x9n[:, :, ::-1],
                                 x9n[:, :, ::-1])

            # ---- epilogue (all 16 row-tiles wide) ----
            ep = sing

            # suffix-sum scan: S[:, :, m] = s_{9-m} (m=1..9), S[:,:,0]=s_9
            S = ep.tile([P, NT, NO], F32)
            nc.vector.tensor_tensor_scan(
                out=S[:].rearrange("p b k -> p (b k)"),
                data0=d0s[:].rearrange("p b k -> p (b k)"),
                data1=d1s[:].rearrange("p b k -> p (b k)"),
                initial=0.0, op0=OP.mult, op1=OP.add,
            )

            # rsqrt: Quake seed + 1 Newton step (err ~4.6e-6)
            sbits = S[:, :, 1:NO].bitcast(I32)
            y0i = ep.tile([P, NT, K], I32)
            nc.vector.tensor_scalar(out=y0i[:], in0=sbits, scalar1=1,
                                    scalar2=-1, op0=OP.arith_shift_right,
                                    op1=OP.bitwise_xor)
            nc.vector.tensor_scalar(out=y0i[:], in0=y0i[:],
                                    scalar1=RSQRT_MAGIC + 1, scalar2=None,
                                    op0=OP.add)
            yv = y0i[:].bitcast(F32)
            # theta slots use the raw Quake seed (rel err <= 1.75e-3 ->
            # theta err ~3e-4 -> softmax norm err ~2e-4, fine); only the
            # r slot (multiplies all logits) gets a Newton step, on [P,NT,1]
            aa8 = ep.tile([P, NT, 1], F32)
            inv8 = ep.tile([P, NT, 1], F32)
            dacc = ep.tile([P, 1], F32)
            nc.vector.tensor_mul(aa8[:], yv[:, :, K - 1:K], yv[:, :, K - 1:K])
            nc.vector.tensor_mul(aa8[:], aa8[:], S[:, :, NO - 1:NO])
            nc.vector.affine_mul_reduce(out=inv8[:], accum_out=dacc[:],
                                        in0=aa8[:], in1=yv[:, :, K - 1:K],
                                        scale=-0.5, bias=1.5)

            # cumprod seed: r3 = c * s_0 * rsqrt(s_0)
            nc.vector.scalar_tensor_tensor(
                out=d1p[:, :, 0:1], in0=S[:, :, NO - 1:NO],
                scalar=pct[:, PC_C:PC_C + 1], in1=inv8[:],
                op0=OP.mult, op1=OP.mult,
            )

            # z = x * seed-rsqrt(s) (reversed order), arcsin series (2 terms)
            z = ep.tile([P, NT, K], F32)
            nc.vector.tensor_mul(z[:], x9n[:, :, ::-1], yv)
            u2 = ep.tile([P, NT, K], F32)
            nc.vector.tensor_mul(u2[:], z[:], z[:])
            w2 = ep.tile([P, NT, K], F32)
            nc.vector.affine_mul_reduce(out=w2[:], accum_out=dacc[:],
                                        in0=u2[:], in1=u2[:], scale=C5,
                                        bias=C3)
            asin = ep.tile([P, NT, K], F32)
            nc.vector.scalar_tensor_tensor(out=asin[:], in0=w2[:], scalar=1.0,
                                           op0=OP.add, op1=OP.mult, in1=z[:])
            # asin = (1 + C3 u2 + C5 u2^2) * z

            def bc(col):  # broadcast pc[col:col+9] over the NT dim
                return (pct[:, col:col + K].unsqueeze(1)
                        .broadcast_to([P, NT, K]))

            # th = max(max(c123 - asin, r23), rot3)
            th = ep.tile([P, NT, K], F32)
            nc.vector.scalar_tensor_tensor(out=th[:], in0=asin[:], scalar=-1.0,
                                           in1=bc(PC_C123), op0=OP.mult,
                                           op1=OP.add)
            nc.vector.tensor_tensor(out=th[:], in0=th[:], in1=bc(PC_R23),
                                    op=OP.max)
            nc.vector.tensor_tensor(out=th[:], in0=th[:], in1=bc(PC_R3),
                                    op=OP.max)

            if range_reduce:
                # range-reduce into [-pi, pi]
                un = ep.tile([P, NT, K], F32)
                nv = ep.tile([P, NT, K], F32)
                nc.vector.tensor_scalar(out=un[:], in0=th[:], scalar1=INV_2PI,
                                        scalar2=MAGIC, op0=OP.mult, op1=OP.add)
                nc.vector.tensor_scalar(out=nv[:], in0=un[:], scalar1=MAGIC,
                                        scalar2=None, op0=OP.subtract)
                thpT = ep.tile([P, NT, K], F32)
                nc.vector.scalar_tensor_tensor(out=thpT[:], in0=nv[:],
                                               scalar=-TWO_PI, in1=th[:],
                                               op0=OP.mult, op1=OP.add)
                thp = thpT[:]
            else:
                thp = th[:]

            # sins (natural order) into scnb slots 1..9; cos via half-angle
            # (the post-sin DVE ops overlap the exp table load on ACT)
            nc.scalar.activation(scnb[:, :, 1:NO], thp[:, :, ::-1], AF.Sin)
            sh = ep.tile([P, NT, K], F32)
            nc.scalar.activation(sh[:], thp, AF.Sin, scale=0.5)
            ccr = ep.tile([P, NT, K], F32)
            nc.vector.tensor_mul(ccr[:], sh[:], sh[:])
            nc.vector.tensor_scalar(out=ccr[:], in0=ccr[:], scalar1=-2.0,
                                    scalar2=1.0, op0=OP.mult, op1=OP.add)

            # cumprod scan: PP[:, :, m] = r3 * prod_{i<m} sin_i
            PP = ep.tile([P, NT, NO], F32)
            nc.vector.tensor_tensor_scan(
                out=PP[:].rearrange("p b k -> p (b k)"),
                data0=scnb[:].rearrange("p b k -> p (b k)"),
                data1=d1p[:].rearrange("p b k -> p (b k)"),
                initial=0.0, op0=OP.mult, op1=OP.add,
            )

            lg = ep.tile([P, NT, NO], F32)
            nc.vector.tensor_mul(lg[:, :, 0:1], PP[:, :, K - 1:K],
                                 ccr[:, :, 0:1])
            nc.vector.tensor_mul(lg[:, :, 1:2], PP[:, :, K - 1:K],
                                 scnb[:, :, NO - 1:NO])
            nc.vector.tensor_mul(lg[:, :, 2:NO], PP[:, :, 7::-1],
                                 ccr[:, :, 1:K])

            # softmax without max-sub (|logits| <= ~45, f32-safe),
            # pipelined in halves so the first output DMA issues early
            E = ep.tile([P, NT, NO], F32)
            ds = ep.tile([P, NT], F32)
            dinv = ep.tile([P, NT], F32)
            out = ep.tile([P, NT, NO], F32)
            H = NT // 2
            for h, (lo, hi) in enumerate(((0, H), (H, NT))):
                w = hi - lo
                nc.scalar.activation(E[:, lo:hi, :], lg[:, lo:hi, :], AF.Exp)
                nc.vector.tensor_reduce(out=ds[:, lo:hi], in_=E[:, lo:hi, :],
                                        axis=mybir.AxisListType.X, op=OP.add)
                nc.vector.reciprocal(dinv[:, lo:hi], ds[:, lo:hi])
                nc.vector.tensor_mul(
                    out[:, lo:hi, :], E[:, lo:hi, :],
                    dinv[:, lo:hi].unsqueeze(2).broadcast_to([P, w, NO]))
                eng = nc.sync if h == 0 else nc.scalar
                eng.dma_start(y_view[:, lo:hi, :], out[:, lo:hi, :])

    nc.compile()
    return nc


_NC = None
_NC_RR = None


def _get_nc(range_reduce):
    global _NC, _NC_RR
    if range_reduce:
        if _NC_RR is None:
            _NC_RR = _build(True)
        return _NC_RR
    if _NC is None:
        _NC = _build(False)
    return _NC


def _host_params(scale1, rot1, scale2, rot2, scale3, rot3):
    c = max(max(float(scale1[0]), 0.0) * float(scale2[0]), 0.0) * float(scale3[0])
    rev = np.arange(8, -1, -1)
    r1 = rot1[:K].astype(np.float64)[rev]
    r2 = rot2[:K].astype(np.float64)[rev]
    r3 = rot3[:K].astype(np.float64)[rev]
    c123 = np.pi / 2 + r1 + r2 + r3
    r23 = r2 + r3
    # per-slot 2*pi*n shift: th = max(c123 - asin, r23, r3) is invariant
    # under shifting all three constants by -2*pi*n (sin period); choose n
    # so the reachable th range lands within [-HOST_SHIFT_LIMIT, +...]
    A = 0.35                               # |asin| bound (actual max ~0.19)
    th_min = np.maximum.reduce([c123 - A, r23, r3])
    th_max = np.maximum.reduce([c123 + A, r23, r3])
    n = np.round((th_min + th_max) / 2 / (2 * np.pi))
    range_reduce = bool(
        (th_max - 2 * np.pi * n > HOST_SHIFT_LIMIT).any()
        or (th_min - 2 * np.pi * n < -HOST_SHIFT_LIMIT).any())
    if not range_reduce:
        c123 = c123 - 2 * np.pi * n
        r23 = r23 - 2 * np.pi * n
        r3 = r3 - 2 * np.pi * n
    row = np.zeros((PC_W,), np.float64)
    row[PC_C] = c
    row[PC_C123:PC_C123 + K] = c123
    row[PC_R23:PC_R23 + K] = r23
    row[PC_R3:PC_R3 + K] = r3
    row[PC_HPI] = np.pi / 2
    return np.tile(row.astype(np.float32)[None, :], (P, 1)), range_reduce


def kernel(x, scale1, rot1, scale2, rot2, scale3, rot3, _trace=False):
    pc, range_reduce = _host_params(scale1, rot1, scale2, rot2, scale3, rot3)
    nc = _get_nc(range_reduce)
    x = np.ascontiguousarray(x, dtype=np.float32)
    x9h = np.ascontiguousarray(x[:, 0:K])
    tail = x[:, K:]                                     # [B, 775]
    t16 = tail.astype(np.float16)
    t8 = tail.astype(ml_dtypes.float8_e4m3fn)
    in_maps = []
    for cidx in range(NCORES):
        m = {"pc": pc, "x9": x9h[cidx * ROWS:(cidx + 1) * ROWS]}
        for g, (t0g, ntg, _q) in enumerate(GROUPS):
            src = t16 if GROUP_F16[g] else t8
            # partition p, slot j -> row 16*p + t0g + j
            blk = src[cidx * ROWS:(cidx + 1) * ROWS].reshape(P, NT, NTAIL)
            m[f"xg{g}"] = np.ascontiguousarray(
                blk[:, t0g:t0g + ntg, :]).reshape(P, ntg * NTAIL)
        in_maps.append(m)
    res = run_bass_kernel_spmd(nc, in_maps, core_ids=list(range(NCORES)),
                               trace=_trace)
    outp = np.concatenate([res.results[c]["y"] for c in range(NCORES)], axis=0)
    if _trace:
        return outp, res
    return outp
